# revision 39
# baseline (speedup 1.0000x reference)
"""Trainium2 Bass kernel for nn_BaselineEdgePredictor (embedding_lookup).

8 cores, data-parallel over edges; per core EC=32768 edges.
Edge i of a core sits at SBUF position (partition i%128, free i//128).

v2 design (vs v1 baseline):
  - emb_type folded into the feature tables host-side (each table belongs to
    exactly one node type; active rows carry emb_type[type]/n_active), so the
    10th gather disappears.
  - One dma_gather per table per chunk covering all THREE endpoints
    (src/dst/neg concatenated, 3*2048+16 indices) -- 9 instead of 30
    gather instructions per chunk, amortizing the ~1us fixed SWDGE cost.
  - x-row fetches batched: 4 indirect DMAs of 8192 rows per endpoint for the
    whole core (vs 768 x 128-row DMAs).
  - Index arithmetic for all 27 (endpoint, table) planes in ~5 DVE ops per
    chunk via broadcast APs.
  - Elementwise work split across DVE / Pool / Act so DMA (~1.26 ms of
    gather traffic) is the only roofline.

pos = relu(h_s+h_d)·out_w + msg·(out_w@edge_w) + (out_w·edge_b + out_b)
"""
import numpy as np

import concourse.bass as bass
import concourse.bacc as bacc
import concourse.mybir as mybir
import concourse.tile as tile
from concourse.bass_utils import run_bass_kernel_spmd

P = 128
EMB = 128
EDGE_DIM = 27
N_TAB = 9
CARD = 50_000
N_NODES = 1_000_000
E_FULL = 262_144
N_CORES = 8
TPC = (0, 0, 0, 0, 1, 2, 2, 2, 2)

STRIDE = CARD + 1                   # zero row at h*STRIDE + CARD
NROWS_STACKED = N_TAB * STRIDE      # 450009
BOFF = 32768

CHUNK = 2048
JT = CHUNK // P                     # 16 free columns per chunk
NEP = 3                             # endpoints per edge (src, dst, neg)
NIDX = NEP * CHUNK + 16             # gather indices incl. 16-zero pad
COLW = NIDX // 16                   # 385 idx columns per table
GCOLS = (NIDX + P - 1) // P         # 49 gather output cols

F32 = mybir.dt.float32
I32 = mybir.dt.int32
I16 = mybir.dt.int16

AOT = mybir.AluOpType
AFT = mybir.ActivationFunctionType
DEBUG_TAPS = False


def _sub(ap: bass.AP, off: int, dims) -> bass.AP:
    """View into an existing [P, free] AP: keep partition dim, replace free dims."""
    return bass.AP(ap.tensor, ap.offset + off, [list(ap.ap[0])] + [list(d) for d in dims])


def _perm_matrices() -> np.ndarray:
    """perm[a] routes V[16a + (x%16), :] -> out partition x under
    matmul(out, lhsT=perm[a], rhs=V): out[x,y] = V[16a + x%16, y]."""
    perms = np.zeros((8, P, P), np.float32)
    for a in range(8):
        for x in range(P):
            perms[a, 16 * a + x % 16, x] = 1.0
    return perms


def build_nc(ec: int):
    n_chunk = ec // CHUNK
    jn = ec // P
    nc = bacc.Bacc("TRN2", num_swdge_queues=4)

    x_d = nc.dram_tensor("x", [N_NODES, 10], I32, kind="ExternalInput")
    stk_d = nc.dram_tensor("stk", [NROWS_STACKED, EMB], F32, kind="ExternalInput")
    src_d = nc.dram_tensor("srcT", [P, jn], I32, kind="ExternalInput")
    dst_d = nc.dram_tensor("dstT", [P, jn], I32, kind="ExternalInput")
    neg_d = nc.dram_tensor("negT", [P, jn], I32, kind="ExternalInput")
    msg_d = nc.dram_tensor("msgT", [P, jn * EDGE_DIM], F32, kind="ExternalInput")
    w2rep_d = nc.dram_tensor("w2rep", [P, EDGE_DIM], F32, kind="ExternalInput")
    outwrep_d = nc.dram_tensor("outwrep", [P, EMB], F32, kind="ExternalInput")
    crep_d = nc.dram_tensor("crep", [P, 1], F32, kind="ExternalInput")
    perm_d = nc.dram_tensor("perms", [8, P, P], F32, kind="ExternalInput")
    pos_d = nc.dram_tensor("pos", [P, jn], F32, kind="ExternalOutput")
    negout_d = nc.dram_tensor("neg", [P, jn], F32, kind="ExternalOutput")
    if DEBUG_TAPS:
        xf_dbg = nc.dram_tensor("xf_dbg", [P, NEP * JT * 10], F32, kind="ExternalOutput")
        v_dbg = nc.dram_tensor("v_dbg", [P, NEP * N_TAB * JT], F32, kind="ExternalOutput")
        ix_dbg = nc.dram_tensor("ix_dbg", [P, N_TAB * COLW], I16, kind="ExternalOutput")
        s_dbg = nc.dram_tensor("s_dbg", [P, GCOLS * EMB], F32, kind="ExternalOutput")

    qctr = [0]

    def next_q():
        q = qctr[0] % 4
        qctr[0] += 1
        return q

    with tile.TileContext(nc) as tc:
        with (
            tc.tile_pool(name="const", bufs=1) as cpool,
            tc.tile_pool(name="work", bufs=2) as wpool,
            tc.tile_pool(name="gath", bufs=3) as gpool,
            tc.tile_pool(name="ssum", bufs=2) as spool,
            tc.tile_pool(name="s2p", bufs=3) as spool2,
            tc.tile_pool(name="hsum", bufs=1) as hpool,
            tc.tile_pool(name="idxp", bufs=3) as ipool,
            tc.tile_pool(name="xrp", bufs=3) as xpool,
            tc.tile_pool(name="psumf", bufs=4, space="PSUM") as fpool,
        ):
            # ---- constants / weights (derived weights precomputed on host) ----
            perm_t = []
            for a in range(8):
                pt = cpool.tile([P, P], F32, tag=f"perm{a}")
                nc.sync.dma_start(out=pt[:], in_=perm_d[a])
                perm_t.append(pt)
            w2rep = cpool.tile([P, EDGE_DIM], F32, tag="w2rep")
            nc.sync.dma_start(out=w2rep[:], in_=w2rep_d[:])
            outwrep = cpool.tile([P, EMB], F32, tag="outwrep")
            nc.sync.dma_start(out=outwrep[:], in_=outwrep_d[:])
            crep = cpool.tile([P, 1], F32, tag="crep")
            nc.sync.dma_start(out=crep[:], in_=crep_d[:])

            # ---- per-edge index arrays + batched x-row gathers ----
            epT = []
            for name, d in (("s", src_d), ("d", dst_d), ("n", neg_d)):
                t = cpool.tile([P, jn], I32, tag=f"{name}T")
                nc.sync.dma_start(out=t[:], in_=d[:])
                epT.append(t)
            posbuf = cpool.tile([P, jn], F32, tag="posbuf")
            negbuf = cpool.tile([P, jn], F32, tag="negbuf")

            # x-row fetches: per-column [P,1]-offset indirect DMAs (the only
            # form the HW executes correctly). One chunk = 3 eps x 16 cols =
            # 48 instructions; ~1us of Pool each, so they are interleaved
            # between the gather descgens (which stall on G-buffer gates).
            xbufs = {}                       # chunk -> per-endpoint tiles

            def xfetch_instrs(c):
                """Return thunks emitting chunk c's x-row fetch instructions."""
                tiles = []
                for ep in range(NEP):
                    xr = xpool.tile([P, JT * 10], I32, tag=f"xr{ep}",
                                    name=f"xr{ep}")
                    tiles.append(xr)
                xbufs[c] = tiles
                thunks = []
                for j in range(JT):
                    for ep in range(NEP):
                        def th(ep=ep, j=j):
                            nc.gpsimd.indirect_dma_start(
                                out=_sub(xbufs[c][ep][:], j * 10, [[1, 10]]),
                                out_offset=None,
                                in_=x_d[:],
                                in_offset=bass.IndirectOffsetOnAxis(
                                    ap=epT[ep][:, c * JT + j:c * JT + j + 1],
                                    axis=0),
                            )
                        thunks.append(th)
                return thunks

            outwb = _sub(outwrep[:], 0, [[0, JT], [1, EMB]])
            pend = {}          # chunk -> (S_all, s2, hs_pos, hs_neg)

            def combine_tail(c):
                """hsum (DVE) + relu (Act) for chunk c, right after adds(c)."""
                S_all, s2 = pend[c]
                Sv = S_all[:]
                S_ep = [_sub(Sv, ep * JT * EMB, [[1, JT * EMB]])
                        for ep in range(NEP)]
                hss = []
                for ep2 in (1, 2):
                    hs = hpool.tile([P, JT * EMB], F32, tag=f"hs{ep2}")
                    nc.vector.tensor_add(out=hs[:], in0=S_ep[0], in1=S_ep[ep2])
                    nc.scalar.activation(out=hs[:], in_=hs[:], func=AFT.Relu)
                    hss.append(hs)
                pend[c] = (S_all, s2, hss)

            def combine_dve(c):
                """dot with out_w + s2 for chunk c (one chunk behind gathers)."""
                S_all, s2, hss = pend.pop(c)
                for hs, obuf in zip(hss, (posbuf, negbuf)):
                    hsv3 = _sub(hs[:], 0, [[EMB, JT], [1, EMB]])
                    nc.vector.tensor_tensor(out=hsv3, in0=hsv3, in1=outwb,
                                            op=AOT.mult)
                    red = wpool.tile([P, JT], F32, tag="rd")
                    nc.vector.tensor_reduce(
                        out=red[:], in_=hsv3,
                        axis=mybir.AxisListType.X, op=AOT.add)
                    nc.vector.tensor_add(out=obuf[:, c * JT:(c + 1) * JT],
                                         in0=red[:], in1=s2[:])

            def prep(c):
                """Index planes + s2 for chunk c; emitted one chunk ahead so
                Pool's fold work precedes the (throttled) descgen stream."""
                j0 = c * JT

                # --- s2 = msg·w2 + c2 (DVE; no gather dependency) ---
                msgc = wpool.tile([P, JT * EDGE_DIM], F32, tag="msgc")
                nc.sync.dma_start(
                    out=msgc[:],
                    in_=bass.AP(msg_d, j0 * EDGE_DIM,
                                [[jn * EDGE_DIM, P], [1, JT * EDGE_DIM]]))
                nc.vector.tensor_tensor(
                    out=msgc[:].rearrange("p (j e) -> p j e", e=EDGE_DIM),
                    in0=msgc[:].rearrange("p (j e) -> p j e", e=EDGE_DIM),
                    in1=_sub(w2rep[:], 0, [[0, JT], [1, EDGE_DIM]]),
                    op=AOT.mult)
                s2 = spool2.tile([P, JT], F32, tag="s2")
                nc.vector.tensor_reduce(
                    out=s2[:], in_=msgc[:].rearrange("p (j e) -> p j e", e=EDGE_DIM),
                    axis=mybir.AxisListType.X, op=AOT.add)
                nc.vector.tensor_tensor(out=s2[:], in0=s2[:],
                                        in1=_sub(crep[:], 0, [[0, JT]]),
                                        op=AOT.add)

                # --- index prep, all on Pool+PE so DVE stays free for adds ---
                # xf: [P, NEP*JT*10] f32 cast of this chunk's x rows, (ep, j, col)
                xf = wpool.tile([P, NEP * JT * 10], F32, tag="xf")
                xtiles = xbufs.pop(c)
                for ep in range(NEP):
                    nc.vector.tensor_copy(
                        out=_sub(xf[:], ep * JT * 10, [[1, JT * 10]]),
                        in_=xtiles[ep][:])
                xfa = xf[:]

                # M_all: [P, NEP*N_TAB*JT] f32, (ep, h, j): 1 iff type(ep)==TPC[h]
                M_all = wpool.tile([P, NEP * N_TAB * JT], F32, tag="M")
                Ma = M_all[:]
                for ep in range(NEP):
                    for cls, h0, nh in ((0, 0, 4), (1, 4, 1), (2, 5, 4)):
                        nc.vector.tensor_scalar(
                            out=_sub(Ma, (ep * N_TAB + h0) * JT,
                                     [[JT, nh], [1, JT]]),
                            in0=_sub(xfa, ep * JT * 10, [[0, nh], [10, JT]]),
                            scalar1=float(cls), scalar2=None, op0=AOT.is_equal)

                # V_all: (x[col h+1] - CARD)*M + (CARD-BOFF)
                V_all = wpool.tile([P, NEP * N_TAB * JT], F32, tag="V")
                Va = V_all[:]
                for ep in range(NEP):
                    nc.vector.scalar_tensor_tensor(
                        out=_sub(Va, ep * N_TAB * JT, [[JT, N_TAB], [1, JT]]),
                        in0=_sub(xfa, ep * JT * 10 + 1, [[1, N_TAB], [10, JT]]),
                        scalar=float(-CARD), op0=AOT.add,
                        in1=_sub(Ma, ep * N_TAB * JT, [[JT, N_TAB], [1, JT]]),
                        op1=AOT.mult)
                nc.vector.tensor_scalar_add(out=Va, in0=Va,
                                            scalar1=float(CARD - BOFF))
                if DEBUG_TAPS and c == 0:
                    nc.sync.dma_start(out=xf_dbg[:], in_=xfa)
                    nc.sync.dma_start(out=v_dbg[:], in_=Va)

                # fold into wrapped-16 i16 gather layout:
                # idxm[pi, h*COLW + ep*128 + j*8 + a] = V[16a + pi%16, (ep,h,j)]
                idxm = ipool.tile([P, N_TAB * COLW], I16, tag="idxm")
                ia = idxm[:]
                nc.vector.memset(_sub(ia, NEP * 128, [[COLW, N_TAB]]), 0)
                for a in range(8):
                    fps = fpool.tile([P, NEP * N_TAB * JT], F32, tag="foldps")
                    nc.tensor.matmul(out=fps[:], lhsT=perm_t[a][:], rhs=V_all[:],
                                     start=True, stop=True)
                    nc.vector.tensor_copy(
                        out=_sub(ia, a, [[128, NEP], [COLW, N_TAB], [8, JT]]),
                        in_=fps[:].rearrange("p (e h j) -> p e h j",
                                             e=NEP, h=N_TAB),
                    )

                prepped[c] = (idxm, s2)

            prepped = {}
            for xth in xfetch_instrs(0):
                xth()
            if n_chunk > 1:
                for xth in xfetch_instrs(1):
                    xth()
            prep(0)
            for c in range(n_chunk):
                if c + 1 < n_chunk:
                    prep(c + 1)
                idxm, s2 = prepped.pop(c)
                ia = idxm[:]

                # --- finish chunk c-2's output while gathers stream ---
                if c >= 2:
                    combine_dve(c - 2)

                # x-row fetches for chunk c+2, interleaved below so they fill
                # Pool's G-buffer gate waits between descgens
                xth = xfetch_instrs(c + 2) if c + 2 < n_chunk else []
                xi, xper = 0, (len(xth) + N_TAB - 1) // N_TAB if xth else 0

                # --- 9 merged gathers; h=0 lands directly in the accumulator ---
                S_all = spool.tile([P, GCOLS * EMB], F32, tag="S")
                for h in range(N_TAB):
                    base = h * STRIDE + BOFF
                    win = stk_d[base:NROWS_STACKED]
                    dst = S_all if h == 0 else gpool.tile(
                        [P, GCOLS * EMB], F32, tag="G")
                    nc.gpsimd.dma_gather(
                        out_ap=dst[:].rearrange("p (g e) -> p g e", e=EMB),
                        in_ap=win,
                        idxs_ap=_sub(ia, h * COLW, [[1, COLW]]),
                        num_idxs=NIDX,
                        num_idxs_reg=NIDX,
                        elem_size=EMB,
                        single_packet=False,
                        queue_num=next_q(),
                    )
                    for _ in range(xper):
                        if xi < len(xth):
                            xth[xi]()
                            xi += 1
                    if h > 0:
                        nc.vector.tensor_add(out=S_all[:], in0=S_all[:],
                                             in1=dst[:])
                while xi < len(xth):
                    xth[xi]()
                    xi += 1

                if DEBUG_TAPS and c == 0:
                    nc.sync.dma_start(out=ix_dbg[:], in_=ia)
                    nc.sync.dma_start(out=s_dbg[:], in_=S_all[:])
                pend[c] = (S_all, s2)
                if c >= 1:
                    combine_tail(c - 1)
            combine_tail(n_chunk - 1)
            if n_chunk >= 2:
                combine_dve(n_chunk - 2)
            combine_dve(n_chunk - 1)

            nc.sync.dma_start(out=pos_d[:], in_=posbuf[:])
            nc.sync.dma_start(out=negout_d[:], in_=negbuf[:])
    nc.compile()
    return nc


def _stage_per_edge(arr: np.ndarray, ec: int, core: int) -> np.ndarray:
    a = arr[core * ec:(core + 1) * ec]
    if a.ndim == 1:
        return np.ascontiguousarray(a.reshape(ec // P, P).T)
    d = a.shape[1]
    return np.ascontiguousarray(
        a.reshape(ec // P, P, d).transpose(1, 0, 2).reshape(P, (ec // P) * d)
    )


def prepare_in_maps(inputs: dict, ec: int) -> list[dict]:
    x, src, dst, neg_dst, msg = (inputs[k] for k in
                                 ("x", "src", "dst", "neg_dst", "msg"))
    emb_type, emb_feats = inputs["emb_type"], inputs["emb_feats"]
    edge_w, edge_b = inputs["edge_w"], inputs["edge_b"]
    out_w, out_b = inputs["out_w"], inputs["out_b"]

    # Stacked tables with emb_type folded in: each table h belongs to node
    # type TPC[h]; active rows carry emb_type[TPC[h]] / n_tables_of_that_type
    # so the per-type sum reconstructs one full emb_type contribution.
    # Zero rows (one per table, at h*STRIDE+CARD) stay zero for inactive hits.
    et = np.asarray(emb_type, np.float32)
    n_per_type = np.bincount(np.asarray(TPC), minlength=3)  # [4, 1, 4]
    stacked = np.zeros((NROWS_STACKED, EMB), np.float32)
    ef = np.asarray(emb_feats, np.float32)
    for h in range(N_TAB):
        t = TPC[h]
        stacked[h * STRIDE:h * STRIDE + CARD] = ef[h] + et[t] / n_per_type[t]

    # derived combine weights: pos = relu(h)·out_w + msg·w2 + c2
    ow = np.asarray(out_w, np.float32).reshape(EMB)
    w2 = ow @ np.asarray(edge_w, np.float32)                 # [EDGE_DIM]
    c2 = float(ow @ np.asarray(edge_b, np.float32).reshape(EMB)
               + np.asarray(out_b, np.float32).reshape(1)[0])
    common = {
        "x": np.ascontiguousarray(np.asarray(x, np.int32)),
        "stk": stacked,
        "w2rep": np.ascontiguousarray(np.broadcast_to(w2, (P, EDGE_DIM))),
        "outwrep": np.ascontiguousarray(np.broadcast_to(ow, (P, EMB))),
        "crep": np.full((P, 1), c2, np.float32),
        "perms": _perm_matrices(),
    }
    in_maps = []
    for c in range(N_CORES):
        in_maps.append(dict(
            common,
            srcT=_stage_per_edge(np.asarray(src, np.int32), ec, c),
            dstT=_stage_per_edge(np.asarray(dst, np.int32), ec, c),
            negT=_stage_per_edge(np.asarray(neg_dst, np.int32), ec, c),
            msgT=_stage_per_edge(np.asarray(msg, np.float32), ec, c),
        ))
    return in_maps


LAST_EXEC_NS = None
LAST_RESULT = None


def _run(x, src, dst, neg_dst, msg, emb_type, emb_feats,
         edge_w, edge_b, out_w, out_b, ec: int, trace: bool = False):
    global LAST_EXEC_NS, LAST_RESULT
    nc = build_nc(ec)
    in_maps = prepare_in_maps(
        dict(x=x, src=src, dst=dst, neg_dst=neg_dst, msg=msg,
             emb_type=emb_type, emb_feats=emb_feats, edge_w=edge_w,
             edge_b=edge_b, out_w=out_w, out_b=out_b), ec)

    res = run_bass_kernel_spmd(nc, in_maps, core_ids=list(range(N_CORES)),
                               trace=trace)
    LAST_EXEC_NS = res.exec_time_ns
    LAST_RESULT = res

    pos = np.empty((N_CORES * ec, 1), np.float32)
    neg = np.empty((N_CORES * ec, 1), np.float32)
    for c in range(N_CORES):
        pos[c * ec:(c + 1) * ec, 0] = res.results[c]["pos"].T.ravel()
        neg[c * ec:(c + 1) * ec, 0] = res.results[c]["neg"].T.ravel()
    return pos, neg


def kernel(x, src, dst, neg_dst, msg, emb_type, emb_feats,
           edge_w, edge_b, out_w, out_b):
    return _run(x, src, dst, neg_dst, msg, emb_type, emb_feats,
                edge_w, edge_b, out_w, out_b, ec=E_FULL // N_CORES)
